# revision 1
# baseline (speedup 1.0000x reference)
"""TRN2 Bass kernel for nn_CausalAttention2Infusion (B=8, N=2048, D=DK=DV=1024).

att_b = softmax(causal(Q_b K_b^T / sqrt(DK))) V_b  with  Q_b = x_b Wq^T, etc.

Sharding: data-parallel over batch — one batch element per NeuronCore (8 cores).
Host pre-transposes inputs (x^T, W^T), folds the 1/sqrt(DK) scale into Wq, and
casts to bf16; the device computes projections and causal flash-style attention
entirely out of SBUF-resident tensors (Q^T, K^T, V resident in bf16).

Self-contained: call kernel(x=..., Wq=..., Wk=..., Wv=...) -> [8, 2048, 1024] f32.
"""
from contextlib import ExitStack

import numpy as np
import ml_dtypes

import concourse.mybir as mybir
import concourse.tile as tile
from concourse import bacc
from concourse.bass_utils import run_bass_kernel_spmd
from concourse.masks import make_causal_mask, make_identity

F32 = mybir.dt.float32
BF16 = mybir.dt.bfloat16
AX = mybir.AxisListType
ALU = mybir.AluOpType
ACTF = mybir.ActivationFunctionType

P = 128
MASK_VAL = -1e30

B, N, D, DK, DV = 8, 2048, 1024, 1024, 1024
N_CORES = 8


def _build_nc(N=N, D=D, DK=DK, DV=DV, mm_dt=BF16):
    assert N % P == 0 and D % P == 0 and DK % P == 0 and DV % P == 0
    nD, nK, nJ = D // P, DK // P, N // P
    n_ib = N // P
    CH = 512

    nc = bacc.Bacc("TRN2", target_bir_lowering=False, debug=False,
                   num_devices=N_CORES)

    def chunks_of(w):
        res, c0 = [], 0
        while c0 < w:
            res.append((c0, min(CH, w - c0)))
            c0 += CH
        return res

    xT = nc.dram_tensor("xT", [D, N], mm_dt, kind="ExternalInput").ap()
    wqT = nc.dram_tensor("wqT", [D, DK], mm_dt, kind="ExternalInput").ap()
    wkT = nc.dram_tensor("wkT", [D, DK], mm_dt, kind="ExternalInput").ap()
    wvT = nc.dram_tensor("wvT", [D, DV], mm_dt, kind="ExternalInput").ap()
    out = nc.dram_tensor("out", [N, DV], F32, kind="ExternalOutput").ap()

    with tile.TileContext(nc) as tc, ExitStack() as ctx:
        resid = ctx.enter_context(tc.tile_pool(name="resid", bufs=1))
        wpool = ctx.enter_context(tc.tile_pool(name="wstream", bufs=2))
        spool = ctx.enter_context(tc.tile_pool(name="sstrip", bufs=2))
        ppool = ctx.enter_context(tc.tile_pool(name="pstrip", bufs=2))
        ptpool = ctx.enter_context(tc.tile_pool(name="ptstrip", bufs=2))
        opool = ctx.enter_context(tc.tile_pool(name="attout", bufs=2))
        stat = ctx.enter_context(tc.tile_pool(name="stats", bufs=4))
        consts = ctx.enter_context(tc.tile_pool(name="consts", bufs=1))
        psS = ctx.enter_context(tc.tile_pool(name="psS", bufs=3, space="PSUM"))
        psT = ctx.enter_context(tc.tile_pool(name="psT", bufs=2, space="PSUM"))
        psA = ctx.enter_context(tc.tile_pool(name="psA", bufs=1, space="PSUM"))

        xt_sb = resid.tile([P, nD, N], mm_dt)
        qt_sb = resid.tile([P, nK, N], mm_dt)
        kt_sb = resid.tile([P, nK, N], mm_dt)
        v_sb = resid.tile([P, nJ, DV], mm_dt)

        ident = consts.tile([P, P], mm_dt)
        cmask = consts.tile([P, P], F32)
        make_identity(nc, ident[:])
        make_causal_mask(nc, cmask[:], mask_val=MASK_VAL)

        nc.sync.dma_start(xt_sb[:], xT.rearrange("(t p) n -> p t n", p=P))

        # phase 1: projections.  K^T/Q^T: stationary = W^T d-slice, moving = x^T
        for w_ap, w_cols, dst in ((wkT, DK, kt_sb), (wqT, DK, qt_sb)):
            w_sb = wpool.tile([P, nD, w_cols], mm_dt, tag="w")
            nc.sync.dma_start(w_sb[:], w_ap.rearrange("(t p) k -> p t k", p=P))
            for kt in range(w_cols // P):
                for ic, (c0, cw) in enumerate(chunks_of(N)):
                    ps = psS.tile([P, CH], F32, tag="sch")
                    for d in range(nD):
                        nc.tensor.matmul(
                            ps[:, :cw],
                            w_sb[:, d, kt * P:(kt + 1) * P],
                            xt_sb[:, d, c0:c0 + cw],
                            start=(d == 0), stop=(d == nD - 1),
                        )
                    if (kt + ic) % 2 == 0:
                        nc.vector.tensor_copy(dst[:, kt, c0:c0 + cw], ps[:, :cw])
                    else:
                        nc.scalar.copy(dst[:, kt, c0:c0 + cw], ps[:, :cw])

        # V natural: stationary = x^T j-slice, moving = Wv^T
        wv_sb = wpool.tile([P, nD, DV], mm_dt, tag="w")
        nc.sync.dma_start(wv_sb[:], wvT.rearrange("(t p) v -> p t v", p=P))
        for jt in range(nJ):
            for vc, (c0, cw) in enumerate(chunks_of(DV)):
                ps = psS.tile([P, CH], F32, tag="sch")
                for d in range(nD):
                    nc.tensor.matmul(
                        ps[:, :cw],
                        xt_sb[:, d, jt * P:(jt + 1) * P],
                        wv_sb[:, d, c0:c0 + cw],
                        start=(d == 0), stop=(d == nD - 1),
                    )
                if (jt + vc) % 2 == 0:
                    nc.vector.tensor_copy(v_sb[:, jt, c0:c0 + cw], ps[:, :cw])
                else:
                    nc.scalar.copy(v_sb[:, jt, c0:c0 + cw], ps[:, :cw])

        # phase 2: causal attention over 128-row strips
        s_strips = {}

        def emit_S(ib):
            w = (ib + 1) * P
            s_sb = spool.tile([P, N], F32, tag="s")
            cmaxs = stat.tile([P, 8], F32, tag="cmax")
            for ci, (c0, cw) in enumerate(chunks_of(w)):
                ps = psS.tile([P, CH], F32, tag="sch")
                for k in range(nK):
                    nc.tensor.matmul(
                        ps[:, :cw],
                        qt_sb[:, k, ib * P:(ib + 1) * P],
                        kt_sb[:, k, c0:c0 + cw],
                        start=(k == 0), stop=(k == nK - 1),
                    )
                if c0 + cw == w:
                    nc.vector.tensor_add(ps[:, cw - P:cw], ps[:, cw - P:cw], cmask[:])
                nc.vector.tensor_reduce(
                    cmaxs[:, ci:ci + 1], ps[:, :cw], axis=AX.X, op=ALU.max)
                nc.scalar.copy(s_sb[:, c0:c0 + cw], ps[:, :cw])
            s_strips[ib] = (s_sb, cmaxs, len(chunks_of(w)))

        def emit_softmax_and_pv(ib):
            w = (ib + 1) * P
            s_sb, cmaxs, nch = s_strips.pop(ib)
            mneg = stat.tile([P, 1], F32, tag="mneg")
            lsum = stat.tile([P, 1], F32, tag="lsum")
            rcp = stat.tile([P, 1], F32, tag="rcp")
            nc.vector.tensor_reduce(
                mneg[:], cmaxs[:, :nch], axis=AX.X, op=ALU.max, negate=True)
            p_sb = ppool.tile([P, N], mm_dt, tag="p")
            nc.scalar.activation(
                p_sb[:, :w], s_sb[:, :w], ACTF.Exp,
                bias=mneg[:], scale=1.0, accum_out=lsum[:])
            nc.vector.reciprocal(rcp[:], lsum[:])

            pt_sb = ptpool.tile([P, nJ, P], mm_dt, tag="pt")
            for jt in range(ib + 1):
                pst = psT.tile([P, P], mm_dt, tag="pt_ps")
                nc.tensor.transpose(pst[:], p_sb[:, jt * P:(jt + 1) * P], ident[:])
                nc.vector.tensor_copy(pt_sb[:, jt], pst[:])
            ps_att = psA.tile([P, DV], F32, tag="att")
            for vc, (c0, cw) in enumerate(chunks_of(DV)):
                for jt in range(ib + 1):
                    nc.tensor.matmul(
                        ps_att[:, c0:c0 + cw],
                        pt_sb[:, jt],
                        v_sb[:, jt, c0:c0 + cw],
                        start=(jt == 0), stop=(jt == ib),
                    )
            o_sb = opool.tile([P, DV], F32, tag="o")
            nc.vector.tensor_scalar_mul(o_sb[:], ps_att[:], rcp[:])
            nc.sync.dma_start(out[ib * P:(ib + 1) * P, :], o_sb[:])

        emit_S(0)
        for ib in range(n_ib):
            if ib + 1 < n_ib:
                emit_S(ib + 1)
            emit_softmax_and_pv(ib)

    nc.compile()
    return nc


_NC_CACHE = {}


def _get_nc():
    if "nc" not in _NC_CACHE:
        _NC_CACHE["nc"] = _build_nc()
    return _NC_CACHE["nc"]


def _host_prepare(x_b, Wq_s, WkT, WvT):
    bf = ml_dtypes.bfloat16
    return {
        "xT": np.ascontiguousarray(x_b.T).astype(bf),
        "wqT": Wq_s,
        "wkT": WkT,
        "wvT": WvT,
    }


def kernel(x, Wq, Wk, Wv):
    x = np.asarray(x, dtype=np.float32)
    Wq = np.asarray(Wq, dtype=np.float32)
    Wk = np.asarray(Wk, dtype=np.float32)
    Wv = np.asarray(Wv, dtype=np.float32)
    assert x.shape == (B, N, D), x.shape

    nc = _get_nc()
    bf = ml_dtypes.bfloat16
    scale = np.float32(1.0) / np.sqrt(np.float32(DK))
    wqT = np.ascontiguousarray((Wq * scale).T).astype(bf)
    wkT = np.ascontiguousarray(Wk.T).astype(bf)
    wvT = np.ascontiguousarray(Wv.T).astype(bf)
    in_maps = [_host_prepare(x[b], wqT, wkT, wvT) for b in range(B)]

    res = run_bass_kernel_spmd(nc, in_maps, list(range(N_CORES)))
    return np.stack([res.results[b]["out"] for b in range(B)], axis=0)
